# revision 23
# baseline (speedup 1.0000x reference)
"""CP-decomposed 3x3 conv on 8 TRN2 NeuronCores.

Math: out[f,i,j] = sum_{h,w,c,r} in[c,i+h,j+w] * f1[h,r] * f2[w,r] * f3[c,r] * f0[f,r]

Factorization used on-device (per core, over its 32 output rows):
  stage A: t2[r, n]  = sum_h sum_c (f3[c,r]*f1[h,r]) * x[c, n + h*W]     (3 matmuls, K=C)
  stage B: out[f, n] = sum_w sum_r (f2[w,r]*f0[f,r]) * t2[r, n + w]      (3 matmuls, K=R)
where n flattens (row, col) with row pitch W=256; output cols 254/255 of each
row are garbage and are dropped at host gather.

Per-core layout (v3): the 32 output rows split into two 16-row halves. SBUF
partitions 0-63 hold half0's input rows [0,18), partitions 64-127 hold half1's
rows [16,34). The host ships x with the 2 halo rows duplicated ([C, 36, W]:
rows 0-17 then 16-33) so a single rearranged DMA covers all 128 partitions.
The input is split into two column-block tiles XA (cols [0,2560)) and XB
([2048,4608)) loaded on the two HWDGE rings (sync/scalar) so compute starts
after ~1/2 of the input landed and loads overlap compute.

Stage A packs 4 matmuls (2 input halves x 2 chunk parities) onto the four
64x64 PE quadrants (tile_position auto-derived from lhsT/psum bases); stage B
runs 2-way on the 64-row groups with M=128, weights grouped so consecutive
same-row-group matmuls share the stationary operand. Chunk strips in t2 are
self-contained: shifted reads spilling past a strip only feed discarded
output columns.

I/O is bf16 both ways; output rows are written 256-wide, 8 rows per DMA,
alternating rings, and trimmed to 254 at host gather.

Sharding: output rows (Ho=254) split across 8 cores: cores 0-6 get rows
[32i, 32i+32); core 7 processes rows [222, 254) via a shifted window (its
first 2 rows duplicate core 6's tail and are dropped at gather).
"""

import sys

sys.path.insert(0, "/opt/trn_rl_repo")

import numpy as np

# Problem constants (hardcoded per contract)
C = 64
H = 256
W = 256
FH = 3
FW = 3
RANK = 64
F = 128
HO = H - FH + 1  # 254
WO = W - FW + 1  # 254
NCORES = 8
ROWS = 32  # output rows per core
IN_ROWS = ROWS + 2  # 34
HALF_OUT = ROWS // 2  # 16 output rows per half
HALF_IN = HALF_OUT + 2  # 18 input rows per half
HCOLS = HALF_IN * W  # 4608 input cols per half
XBLK = 2560  # cols per X block tile; XB starts at 2048
CHUNK = 512  # output elements per chunk (= 2 rows x 256)
NQUAD = 4  # quad-iters; each covers 2 chunks per half (4 rows per half)

COMPUTE_DT = "bf16"
SPLIT_EVAC = False
HALF_BLOCK_B = True
# Ablation switches for benchmarking: subset of
# {"in_dma", "out_dma", "stage_a", "stage_b", "copies"}
ABLATE = set()

_PROGRAM_CACHE = {}


def _np_compute_dtype():
    import ml_dtypes

    if COMPUTE_DT == "fp16":
        return np.dtype(ml_dtypes.float16)
    return np.dtype(ml_dtypes.bfloat16)


def build_program(
    rows=ROWS,
    compute_dt=None,
    num_devices=NCORES,
    reps=1,
    paired=None,  # unused; kept for bench.py compat
    bench_internal=False,
):
    """Build + compile the per-core Bass program."""
    from concourse import bacc, mybir, tile

    compute_dt = compute_dt or COMPUTE_DT
    dt_c = mybir.dt.float16 if compute_dt == "fp16" else mybir.dt.bfloat16
    dt_f32 = mybir.dt.float32

    assert rows == ROWS

    nc = bacc.Bacc(
        "TRN2", target_bir_lowering=False, debug=False, num_devices=num_devices
    )
    if bench_internal:
        x = nc.dram_tensor("x_int", [2 * C, HCOLS], dt_c).ap()
        wa2 = nc.dram_tensor("wa2_int", [2 * C, FH * RANK], dt_c).ap()
        wb2 = nc.dram_tensor("wb2_int", [2 * RANK, FW * F], dt_c).ap()
        y = nc.dram_tensor("y_int", [F, ROWS, W], dt_c).ap()
        tin = nc.dram_tensor("tin", [1, 16], dt_f32, kind="ExternalInput").ap()
        tout = nc.dram_tensor("tout", [1, 16], dt_f32, kind="ExternalOutput").ap()
    else:
        x = nc.dram_tensor("x", [2 * C, HCOLS], dt_c, kind="ExternalInput").ap()
        wa2 = nc.dram_tensor("wa2", [2 * C, FH * RANK], dt_c, kind="ExternalInput").ap()
        wb2 = nc.dram_tensor("wb2", [2 * RANK, FW * F], dt_c, kind="ExternalInput").ap()
        y = nc.dram_tensor("y", [F, ROWS, W], dt_c, kind="ExternalOutput").ap()

    with tile.TileContext(nc) as tc:
        with (
            tc.tile_pool(name="xin", bufs=2) as xin_pool,
            tc.tile_pool(name="wgt", bufs=2) as wgt_pool,
            tc.tile_pool(name="t2", bufs=2) as t2_pool,
            tc.tile_pool(name="ot", bufs=2) as ot_pool,
            tc.tile_pool(name="p1", bufs=2, space="PSUM") as p1_pool,
            tc.tile_pool(name="p2", bufs=2, space="PSUM") as p2_pool,
        ):

            def body():
                # (g c) partition layout: partitions 0-63 = half0 rows 0-17,
                # 64-127 = half1 rows 16-33 (host duplicates the halo rows).
                XA = xin_pool.tile([2 * C, XBLK], dt_c, tag="xa")
                XB = xin_pool.tile([2 * C, XBLK], dt_c, tag="xb")
                WA = wgt_pool.tile([2 * C, FH * RANK], dt_c, tag="wa")
                WB = wgt_pool.tile([2 * RANK, FW * F], dt_c, tag="wb")
                nc.sync.dma_start(out=WA[:], in_=wa2[:])
                nc.scalar.dma_start(out=WB[:], in_=wb2[:])
                if "in_dma" in ABLATE:
                    nc.vector.memset(XA[:, 0:8], 0.0)
                    nc.vector.memset(XB[:, 0:8], 0.0)
                else:
                    nc.sync.dma_start(out=XA[:, 0:1536], in_=x[:, 0:1536])
                    nc.sync.dma_start(out=XA[:, 1536:XBLK], in_=x[:, 1536:XBLK])
                    nc.scalar.dma_start(out=XB[:], in_=x[:, HCOLS - XBLK : HCOLS])

                def xslice(l, h):
                    # chunk l tap h: global cols l*512 + h*W, width 512
                    base = l * CHUNK + h * W
                    if l >= 4:
                        return XB, base - (HCOLS - XBLK)
                    return XA, base

                def stage_a(q):
                    # psum slots (pa, col ca): pa = chunk parity, ca = half.
                    # Quadrant (64*half, pa): all four distinct -> 4-way.
                    p1q = p1_pool.tile([2 * C, 2 * CHUNK], dt_f32)
                    if "stage_a" in ABLATE:
                        nc.vector.memset(p1q[:, 0:8], 0.0)
                    else:
                        for h in range(FH):
                            for half, pa in ((0, 0), (0, 64), (1, 0), (1, 64)):
                                l = 2 * q + (1 if pa else 0)
                                xt, base = xslice(l, h)
                                nc.tensor.matmul(
                                    out=p1q[
                                        pa : pa + RANK,
                                        half * CHUNK : (half + 1) * CHUNK,
                                    ],
                                    lhsT=WA[
                                        half * C : (half + 1) * C,
                                        h * RANK : (h + 1) * RANK,
                                    ],
                                    rhs=xt[
                                        half * C : (half + 1) * C,
                                        base : base + CHUNK,
                                    ],
                                    start=(h == 0),
                                    stop=(h == FH - 1),
                                    skip_group_check=True,
                                )
                    t2q = t2_pool.tile([2 * RANK, 2 * CHUNK + 4], dt_c, tag="t2")
                    if "copies" in ABLATE:
                        nc.vector.memset(t2q[:, 0:8], 0.0)
                    else:
                        # pad cols [1024:1028] stay unwritten: shifted reads
                        # spilling there only feed discarded output columns
                        nc.vector.tensor_copy(out=t2q[:, 0 : 2 * CHUNK], in_=p1q[:])
                    return t2q

                ot_tiles = {}  # (p, half) -> ot tile, stored after odd qi

                def b_mms(p2t, t2q, half, pa_list):
                    for w in range(FW):
                        for pa in pa_list:
                            nc.tensor.matmul(
                                out=p2t[
                                    :, (pa // 64) * CHUNK : (pa // 64 + 1) * CHUNK
                                ],
                                lhsT=WB[pa : pa + RANK, w * F : (w + 1) * F],
                                rhs=t2q[
                                    pa : pa + RANK,
                                    half * CHUNK + w : (half + 1) * CHUNK + w,
                                ],
                                start=(w == 0),
                                stop=(w == FW - 1),
                                skip_group_check=True,
                            )

                def stage_b(q, t2q):
                    # t2q[pa:pa+64, half*512:+512] holds chunk (half, 2q+pa/64)
                    # -> p2q[half] cols (pa/64)*512 -> y rows half*16+4q..+4.
                    p2q = [None, None]
                    if not HALF_BLOCK_B:
                        p2q_a = p2_pool.tile([F, 2 * CHUNK], dt_f32, tag="p2")
                        p2q_b = p2_pool.tile([F, 2 * CHUNK], dt_f32, tag="p2")
                        p2q[0], p2q[1] = p2q_a, p2q_b
                        if "stage_b" in ABLATE:
                            nc.vector.memset(p2q[0][:, 0:8], 0.0)
                            nc.vector.memset(p2q[1][:, 0:8], 0.0)
                        else:
                            # interleave halves: same-row-group matmuls adjacent
                            # share the stationary operand
                            for w in range(FW):
                                for pa in (0, 64):
                                    for half in (0, 1):
                                        nc.tensor.matmul(
                                            out=p2q[half][
                                                :,
                                                (pa // 64) * CHUNK : (pa // 64 + 1)
                                                * CHUNK,
                                            ],
                                            lhsT=WB[
                                                pa : pa + RANK, w * F : (w + 1) * F
                                            ],
                                            rhs=t2q[
                                                pa : pa + RANK,
                                                half * CHUNK
                                                + w : (half + 1) * CHUNK
                                                + w,
                                            ],
                                            start=(w == 0),
                                            stop=(w == FW - 1),
                                            skip_group_check=True,
                                        )
                    if "out_dma" not in ABLATE or HALF_BLOCK_B:
                        yf = y.rearrange("f r w -> f (r w)")
                        p, sub = q // 2, q % 2
                        for half in range(2):
                            if HALF_BLOCK_B:
                                # per-half matmul block: this tile finishes
                                # accumulating while the other half's block
                                # runs, so its evac overlaps PE work
                                p2q_h = p2_pool.tile(
                                    [F, 2 * CHUNK], dt_f32, tag="p2"
                                )
                                p2q[half] = p2q_h
                                if "stage_b" in ABLATE:
                                    nc.vector.memset(p2q[half][:, 0:8], 0.0)
                                else:
                                    b_mms(p2q[half], t2q, half, (0, 64))
                                if "out_dma" in ABLATE:
                                    nc.vector.memset(p2q[half][:, 8:16], 1.0)
                                    continue
                            if sub == 0:
                                ot_new = ot_pool.tile(
                                    [F, 4 * CHUNK], dt_c, tag=f"ot{half}"
                                )
                                ot_tiles[(p, half)] = ot_new
                            ot = ot_tiles[(p, half)]
                            dst = ot[:, sub * 2 * CHUNK : (sub + 1) * 2 * CHUNK]
                            if SPLIT_EVAC or q == NQUAD - 1:
                                # split each evac across both engines: half the
                                # latency on the p2-buffer-reuse critical edge
                                lo, hi = (0, CHUNK) if half == 0 else (CHUNK, 0)
                                nc.vector.tensor_copy(
                                    out=dst[:, lo : lo + CHUNK],
                                    in_=p2q[half][:, lo : lo + CHUNK],
                                )
                                nc.scalar.copy(
                                    out=dst[:, hi : hi + CHUNK],
                                    in_=p2q[half][:, hi : hi + CHUNK],
                                )
                            elif (q + half) % 2 == 0:
                                nc.vector.tensor_copy(out=dst, in_=p2q[half][:])
                            else:
                                nc.scalar.copy(out=dst, in_=p2q[half][:])
                            # flat column slices keep DMA descriptors at 128x4KB
                            r0 = half * HALF_OUT + 8 * p
                            eng = nc.sync if half == 0 else nc.scalar
                            if p == 1 and sub == 1:
                                # very last rows: 2-row stores chase the two
                                # split-evac halves to minimize the tail
                                eng.dma_start(
                                    out=yf[:, (r0 + 4) * W : (r0 + 6) * W],
                                    in_=dst[:, 0:CHUNK],
                                )
                                eng.dma_start(
                                    out=yf[:, (r0 + 6) * W : (r0 + 8) * W],
                                    in_=dst[:, CHUNK : 2 * CHUNK],
                                )
                            elif p == 1:
                                # final row block: store each 4-row sub-block as
                                # soon as its evac lands to shorten the tail
                                eng.dma_start(
                                    out=yf[
                                        :,
                                        (r0 + 4 * sub) * W : (r0 + 4 * (sub + 1)) * W,
                                    ],
                                    in_=dst,
                                )
                            elif sub == 1:
                                eng.dma_start(
                                    out=yf[:, r0 * W : (r0 + 8) * W], in_=ot[:]
                                )

                pending = None
                for q in range(NQUAD + 1):
                    t2q = stage_a(q) if q < NQUAD else None
                    if pending is not None:
                        stage_b(pending[0], pending[1])
                    pending = (q, t2q) if t2q is not None else None

            if reps == 1:
                body()
            else:
                with tc.For_i(0, reps, 1):
                    body()
            if bench_internal:
                nc.sync.dma_start(out=tout[:], in_=tin[:])

    nc.compile()
    return nc


def _get_program():
    key = (ROWS, COMPUTE_DT)
    if key not in _PROGRAM_CACHE:
        _PROGRAM_CACHE[key] = build_program()
    return _PROGRAM_CACHE[key]


def make_weight_inputs(factor0, factor1, factor2, factor3, np_dt=None):
    np_dt = np_dt or _np_compute_dtype()
    f0 = np.asarray(factor0, np.float32)
    f1 = np.asarray(factor1, np.float32)
    f2 = np.asarray(factor2, np.float32)
    f3 = np.asarray(factor3, np.float32)
    # wa[c, h*RANK+r] = f3[c,r] * f1[h,r], duplicated into both halves
    wa = (f3[:, None, :] * f1[None, :, :]).reshape(C, FH * RANK)
    wa2 = np.concatenate([wa, wa], axis=0).astype(np_dt)
    # wb[r, w*F+f] = f2[w,r] * f0[f,r], duplicated into both halves
    wb = (f2.T[:, :, None] * f0.T[:, None, :]).reshape(RANK, FW * F)
    wb2 = np.concatenate([wb, wb], axis=0).astype(np_dt)
    return np.ascontiguousarray(wa2), np.ascontiguousarray(wb2)


ROW_STARTS = [0, 32, 64, 96, 128, 160, 192, 222]


def kernel(input, factor0, factor1, factor2, factor3):
    from concourse.bass_utils import run_bass_kernel_spmd

    nc = _get_program()
    np_dt = _np_compute_dtype()
    wa2, wb2 = make_weight_inputs(factor0, factor1, factor2, factor3, np_dt)
    inp = np.asarray(input, np.float32).astype(np_dt)
    in_maps = []
    for s in ROW_STARTS:
        xs = inp[:, s : s + IN_ROWS, :]
        # partitions (g c): half0 rows 0-17, half1 rows 16-33 -> [2C, 18*W]
        xd = np.stack(
            [xs[:, 0:HALF_IN, :], xs[:, HALF_OUT:IN_ROWS, :]], axis=0
        ).reshape(2 * C, HCOLS)
        in_maps.append(
            {"x": np.ascontiguousarray(xd), "wa2": wa2, "wb2": wb2}
        )
    res = run_bass_kernel_spmd(nc, in_maps, list(range(NCORES))).results
    out = np.empty((F, HO, WO), np.float32)
    for i, s in enumerate(ROW_STARTS):
        ys = res[i]["y"][:, :, 0:WO].astype(np.float32)
        if i < NCORES - 1:
            out[:, s : s + ROWS, :] = ys
        else:
            out[:, 224:HO, :] = ys[:, 2:ROWS, :]
    return out
